# revision 4
# baseline (speedup 1.0000x reference)
"""Trainium2 Bass kernel for CausalSequenceCML.

Math (reference, per step, grid g laid out (B, C, T)):
    mapped  = r * g * (1 - g)
    local   = causal depthwise conv1d(mapped, K, left pad 3)   # per channel
    physics = (1 - eps) * mapped + eps * local
    g'      = (1 - beta) * physics + beta * x0                 # x0 = initial grid

Because r, eps, beta, K are per-channel constants and the conv is linear,
the whole update is affine in a = g*(1-g) = 0.25 - (g-0.5)^2:

    g' = D - C3*sq[t] - C2*sq[t-1] - C1*sq[t-2] - C0*sq[t-3]
    sq = (g - 0.5)^2
    Cj = (1-beta)*eps*r*K[j]             (j = 0, 1, 2)
    C3 = (1-beta)*r*((1-eps) + eps*K[3])
    D  = beta*x0 + 0.25*(C0+C1+C2+C3)

Left-boundary: conv pads mapped (=> a) with zeros, i.e. sq = 0.25 there; the
sq buffer has 3 leading pad columns held at 0.25.

Sharding: C=512 split across 8 cores (64 ch each). Per core the (B=4, 64, T)
block is flattened to 256 rows = 2 SBUF tiles of (128, 2+T), channels+batch
on partitions, time on the free dim (2 leading scratch cols, see below).

Engine split, per step per tile (all fp32 — the chaotic map amplifies
rounding ~3e4x over 16 steps, so 16-bit or float32r anywhere fails):
 - ScalarE: sq = Square(g - 0.5) into the padded sq buffer.
 - VectorE: columns [0, PE_SPLIT) via TWO custom-DVE FIR2 instructions
   (hand-authored uOp program FIR2_ANT):
       E  = D - C3*(sq[t]   + (C1/C3)*sq[t-2])
       g' = E - C2*(sq[t-1] + (C0/C2)*sq[t-3])
   Each runs at ~1 elem/cycle and computes 2 taps + merge: the even-shift
   tap reads the instruction's own input stream 2 elements back through the
   DVE datapath's cross-element flops (NEXT_ALU_OUT_B = next block's
   B-flop, which on TRN2 holds that block's result 2 elements earlier —
   measured on HW). The odd taps come from a second instruction whose
   input AP starts one column earlier. The first 2 outputs of each
   instruction are warmup garbage, absorbed by 2 leading scratch columns
   on the E/g buffers. This replaces 4 scalar_tensor_tensor ops (the
   per-element cost drops 4 -> 2 DVE cycles).
 - TensorE: columns [PE_SPLIT, T) via 5 PSUM-accumulated fp32 matmuls per
   512-col block: W = diag(-Cj) for the 4 taps (time shifts come free via
   the moving operand's AP column offset) plus an identity-diagonal matmul
   that adds D — so PSUM holds g' directly (fp32 matmul is 2-pass LO/HI,
   measured exact to 1e-7).
 - ScalarE copies the finished PSUM blocks to the state buffer.
GPSIMD stays idle: its SBUF port is an exclusive lock shared with DVE's
2-port ops, so concurrent GPSIMD work measured 2.8x slower overall.
"""

import copy

import numpy as np

from concourse import bacc, mybir
import concourse.tile as tile
import concourse.dve_ops as _dve_ops_mod
from concourse.bass_utils import run_bass_kernel_spmd
from concourse.dve_spec import Spec, Src0, Src1, C0 as _C0, C1 as _C1, lower as _dve_lower
from concourse.dve_uop import AluInp, AluOp, DelayInp, DveOpSpec

B, T, C = 4, 4096, 512
N_CORES = 8
CPC = C // N_CORES          # channels per core = 64
ROWS = B * CPC              # 256 rows per core
HALVES = ROWS // 128        # 2 SBUF tiles per core
CLAMP = 1e-4
F32 = mybir.dt.float32
PAD = 48                    # scratch lead cols on g/D/E (absorbs DVE pipe-fill skew)
SQPAD = PAD + 1             # sq lead cols (3 math pads + fill-warmup, all 0.25)

# PE offload: TensorE computes all 4 taps + D for columns [PE_SPLIT, T).
PE_SPLIT = 3264
PE_BLOCK = 512
PE_ADD_D = True

_compiled = {}


# --- custom DVE op: FIR2_ANT ------------------------------------------------
# out[k] = in1[k] - s0*(in0[k] + s1*in0[k-2])
# Built by hand at the uOp level (the Spec DSL has no delay primitive).
# Verified on HW: exact vs fp32 reference for all k >= 2.

class _HandDveOp:
    """Duck-types concourse.dve_ops.DveOp for _custom_dve + table-gen."""

    def __init__(self, name, spec, uops, rd1):
        self.name = name
        self.spec = spec
        self.uops = uops
        self.rd1 = rd1
        self.subdim = False

    def compile(self, ver):
        assert ver == "v3", "FIR2_ANT authored for TRN2/v3"
        return DveOpSpec(
            name=self.name,
            opcode=_dve_ops_mod.get_dve_sub_opcode(self.name),
            uops=list(self.uops),
            rd1_en=self.rd1,
        )


_fir2_op = None


def _get_fir2():
    global _fir2_op
    if _fir2_op is not None:
        return _fir2_op
    existing = next((o for o in _dve_ops_mod.OPS if o.name == "FIR2_ANT"), None)
    if existing is not None:
        _fir2_op = existing
        return existing
    spec = Spec(
        body=Src1 - (Src0 + Src0 * _C1) * _C0,
        reference=lambda in0, in1, s0, s1, imm2: in1 - (in0 + in0 * s1) * s0,
    )
    uops = _dve_lower(spec, ver="v3")
    assert len(uops) == 1
    u = copy.deepcopy(uops[0])
    chain = {}
    for j, s in enumerate(u.inp):
        if u.inp_enable[j]:
            chain[s.name] = j - 1
    x = AluInp.PREV_DELAY_0 + chain["SRC_0"]
    d = AluInp.PREV_DELAY_0 + chain["SRC_1"]
    c0 = AluInp.PREV_DELAY_0 + chain["CONST_0"]
    c1 = AluInp.PREV_DELAY_0 + chain["CONST_1"]
    used = set(chain.values())
    free = next(l for l in range(6) if l not in used)
    m_lane = AluInp.PREV_DELAY_0 + free

    dp = u.datapath_config
    # blk0: m = s1 * in0[k-2]  (NEXT_ALU_OUT_B = blk1's B-flop = blk1's
    # result 2 elements back)
    dp[0].op = AluOp.MULTIPLY
    dp[0].alu_src0 = AluInp.NEXT_ALU_OUT_B
    dp[0].alu_src1 = c1
    dp[0].alu_out_enable = 1
    # blk1: result = in0[k] (bypass); broadcast into B-flop; carry m on a
    # free delay lane
    dp[1].op = AluOp.BYPASS
    dp[1].alu_src0 = x
    dp[1].alu_src1 = x
    dp[1].alu_out_enable = 1
    dp[1].alu_out_b_enable = 1
    dp[1].delay[free] = DelayInp.PREV_ALU_OUT
    dp[1].delay_enable[free] = 1
    # blk2: u = in0[k] + m
    dp[2].op = AluOp.ADD
    dp[2].alu_src0 = AluInp.PREV_ALU_OUT
    dp[2].alu_src1 = m_lane
    dp[2].alu_out_enable = 1
    # blk3: v = s0 * u
    dp[3].op = AluOp.MULTIPLY
    dp[3].alu_src0 = AluInp.PREV_ALU_OUT
    dp[3].alu_src1 = c0
    dp[3].alu_out_enable = 1
    # blk4: out = in1 - v
    dp[4].op = AluOp.SUBTRACT
    dp[4].alu_src0 = d
    dp[4].alu_src1 = AluInp.PREV_ALU_OUT
    dp[4].alu_out_enable = 1
    for b in range(5, 8):
        dp[b].op = AluOp.BYPASS
        dp[b].alu_src0 = AluInp.PREV_ALU_OUT
        dp[b].alu_src1 = AluInp.PREV_ALU_OUT
        dp[b].alu_out_enable = 1
        dp[b].alu_out_a_enable = 0
        dp[b].alu_out_b_enable = 0

    op = _HandDveOp("FIR2_ANT", spec, (u,), rd1=True)
    _dve_ops_mod.OPS.append(op)
    row = _dve_ops_mod._CUSTOM_DVE_ROW_BASE + len(_dve_ops_mod.OPS) - 1
    assert row < 0x20
    _dve_ops_mod._SUB_OPCODE_FOR_NAME[op.name] = row
    _dve_ops_mod.CUSTOM_DVE_SPECS[op.name] = op.spec
    _fir2_op = op
    return op


# --- kernel build -----------------------------------------------------------

def _build(steps: int, loop_k: int | None = None, pe_split: int | None = None,
           pe_add_d: bool | None = None):
    PE_SPLIT = globals()["PE_SPLIT"] if pe_split is None else pe_split
    PE_ADD_D = globals()["PE_ADD_D"] if pe_add_d is None else pe_add_d
    pe_on = PE_SPLIT < T
    fir2 = _get_fir2()
    nc = bacc.Bacc("TRN2", target_bir_lowering=False, debug=False)

    x = nc.dram_tensor("x", [ROWS, T], F32, kind="ExternalInput").ap()
    coef = nc.dram_tensor("coef", [ROWS, 6], F32, kind="ExternalInput").ap()
    out = nc.dram_tensor("out", [ROWS, T], F32, kind="ExternalOutput").ap()
    if pe_on:
        wcols = 640 if PE_ADD_D else 512
        wdiag = nc.dram_tensor("wdiag", [ROWS, wcols], F32,
                               kind="ExternalInput").ap()
        wdiag_h = wdiag.rearrange("(h p) c -> h p c", p=128)

    x_h = x.rearrange("(h p) t -> h p t", p=128)
    out_h = out.rearrange("(h p) t -> h p t", p=128)
    coef_h = coef.rearrange("(h p) c -> h p c", p=128)

    mult = mybir.AluOpType.mult
    add = mybir.AluOpType.add

    with tile.TileContext(nc) as tc:
        with tc.tile_pool(name="state", bufs=1) as pool, \
             tc.tile_pool(name="psum", bufs=8, space="PSUM") as pspool:
            neg_half = pool.tile([128, 1], F32, tag="neg_half", name="neg_half")
            nc.vector.memset(neg_half[:], -0.5)
            gA, gB, sq, D, E, cf = [], [], [], [], [], []
            for h in range(HALVES):
                gA.append(pool.tile([128, PAD + T], F32, tag=f"gA{h}", name=f"gA{h}"))
                gB.append(pool.tile([128, PAD + T], F32, tag=f"gB{h}", name=f"gB{h}"))
                sq.append(pool.tile([128, SQPAD + T], F32, tag=f"sq{h}", name=f"sq{h}"))
                D.append(pool.tile([128, PAD + T], F32, tag=f"D{h}", name=f"D{h}"))
                E.append(pool.tile([128, PAD + T], F32, tag=f"E{h}", name=f"E{h}"))
                cf.append(pool.tile([128, 6], F32, tag=f"cf{h}", name=f"cf{h}"))

            wd = []
            if pe_on:
                for h in range(HALVES):
                    wd.append(pool.tile([128, wcols], F32, tag=f"wd{h}",
                                        name=f"wd{h}"))
                    nc.sync.dma_start(out=wd[h][:], in_=wdiag_h[h])
            for h in range(HALVES):
                nc.sync.dma_start(out=cf[h][:], in_=coef_h[h])
                nc.sync.dma_start(out=gA[h][:, PAD:PAD + T], in_=x_h[h])
                # pad columns stay at a^2-of-zero = 0.25 forever
                nc.vector.memset(sq[h][:, 0:SQPAD], 0.25)
                # scratch lead cols: keep finite (warmup garbage lands here)
                nc.vector.memset(gA[h][:, 0:PAD], 0.0)
                nc.vector.memset(gB[h][:, 0:PAD], 0.0)
                nc.vector.memset(D[h][:, 0:PAD], 0.0)
                # D = beta * x0 + dconst
                nc.vector.tensor_scalar(
                    D[h][:, PAD:PAD + T], gA[h][:, PAD:PAD + T],
                    cf[h][:, 4:5], cf[h][:, 5:6], mult, add,
                )

            dve_end = PE_SPLIT if pe_on else T
            pe_blocks = []
            c = PE_SPLIT
            while c < T:
                n = min(PE_BLOCK, T - c)
                pe_blocks.append((c, n))
                c += n

            def emit_steps():
                for s in range(steps):
                    cur, nxt = (gA, gB) if s % 2 == 0 else (gB, gA)
                    for h in range(HALVES):
                        nc.scalar.activation(
                            sq[h][:, SQPAD:SQPAD + T], cur[h][:, PAD:PAD + T],
                            mybir.ActivationFunctionType.Square,
                            bias=neg_half[:],
                        )
                    # PE region: psum accumulates -Cj taps (+D), ScalarE
                    # copies g' out
                    step_psums = []
                    if pe_on:
                        n_mm = 5 if PE_ADD_D else 4
                        for h in range(HALVES):
                            for (c0, n) in pe_blocks:
                                ps = pspool.tile([128, PE_BLOCK], F32, tag="ps",
                                                 name=f"ps{s}_{h}_{c0}")
                                for k in range(4):
                                    off = SQPAD - k
                                    nc.tensor.matmul(
                                        ps[:, :n],
                                        wd[h][:, k * 128:(k + 1) * 128],
                                        sq[h][:, off + c0:off + c0 + n],
                                        start=(k == 0), stop=(k == n_mm - 1),
                                    )
                                if PE_ADD_D:
                                    nc.tensor.matmul(
                                        ps[:, :n], wd[h][:, 512:640],
                                        D[h][:, PAD + c0:PAD + c0 + n],
                                        start=False, stop=True,
                                    )
                                step_psums.append((h, c0, n, ps))
                    # DVE region: two FIR2 ops per tile.
                    # Output position j <-> real col t = j - PAD; the first
                    # ~25 outputs are pipe-fill garbage (tap offset settles
                    # to exactly 2-back only after the fill phase), absorbed
                    # by PAD scratch lead cols; lead pads are constant 0.25
                    # so skewed warm-up taps read identical values.
                    w = PAD + dve_end
                    for h in range(HALVES):
                        # E = D - C3*(sq[t] + (C1/C3)*sq[t-2])
                        nc.vector._custom_dve(
                            fir2, out=E[h][:, 0:w], in0=sq[h][:, 1:1 + w],
                            in1=D[h][:, 0:w],
                            s0=cf[h][:, 0:1], s1=cf[h][:, 1:2],
                        )
                        # g' = E - C2*(sq[t-1] + (C0/C2)*sq[t-3])
                        nc.vector._custom_dve(
                            fir2, out=nxt[h][:, 0:w], in0=sq[h][:, 0:w],
                            in1=E[h][:, 0:w],
                            s0=cf[h][:, 2:3], s1=cf[h][:, 3:4],
                        )
                    for (h, c0, n, ps) in step_psums:
                        if PE_ADD_D:
                            nc.scalar.copy(nxt[h][:, PAD + c0:PAD + c0 + n],
                                           ps[:, :n])
                        else:
                            nc.vector.scalar_tensor_tensor(
                                nxt[h][:, PAD + c0:PAD + c0 + n], ps[:, :n],
                                -1.0, D[h][:, PAD + c0:PAD + c0 + n],
                                mult, add,
                            )

            if loop_k is not None:
                with tc.For_i(0, loop_k):
                    emit_steps()
            else:
                emit_steps()

            fin = gA if steps % 2 == 0 else gB
            for h in range(HALVES):
                nc.vector.tensor_scalar(
                    fin[h][:, PAD:PAD + T], fin[h][:, PAD:PAD + T],
                    CLAMP, 1.0 - CLAMP,
                    mybir.AluOpType.max, mybir.AluOpType.min,
                )
                nc.sync.dma_start(out=out_h[h], in_=fin[h][:, PAD:PAD + T])

    nc.compile()
    return nc


def get_nc(steps: int):
    if steps not in _compiled:
        _compiled[steps] = _build(steps)
    return _compiled[steps]


def _host_prep(drive, r, eps, beta, K_causal):
    """Per-core input maps: x (256, T), coef (256, 6), wdiag (256, 640)."""
    drive = np.asarray(drive, np.float32)
    r = np.asarray(r, np.float32)
    eps = np.asarray(eps, np.float32)
    beta = np.asarray(beta, np.float32)
    K = np.asarray(K_causal, np.float32)[:, 0, :]  # (C, 4)

    one_m_b = 1.0 - beta
    C0 = one_m_b * eps * r * K[:, 0]
    C1 = one_m_b * eps * r * K[:, 1]
    C2 = one_m_b * eps * r * K[:, 2]
    C3 = one_m_b * r * ((1.0 - eps) + eps * K[:, 3])
    dconst = 0.25 * (C0 + C1 + C2 + C3)

    pe_on = PE_SPLIT < T
    in_maps = []
    idx = np.arange(128)
    for i in range(N_CORES):
        sl = slice(i * CPC, (i + 1) * CPC)
        xs = np.ascontiguousarray(
            drive[:, :, sl].transpose(0, 2, 1).reshape(ROWS, T), np.float32
        )
        cs = np.stack(
            [np.tile(C3[sl], B), np.tile(C1[sl] / C3[sl], B),
             np.tile(C2[sl], B), np.tile(C0[sl] / C2[sl], B),
             np.tile(beta[sl], B), np.tile(dconst[sl], B)],
            axis=1,
        ).astype(np.float32)
        m = {"x": xs, "coef": np.ascontiguousarray(cs)}
        if pe_on:
            sign = -1.0 if PE_ADD_D else 1.0
            blocks = [sign * C3, sign * C2, sign * C1, sign * C0]
            if PE_ADD_D:
                blocks.append(np.ones(C, np.float32))
            wdg = np.zeros((ROWS, 128 * len(blocks)), np.float32)
            for k, arr in enumerate(blocks):
                rows = np.tile(np.asarray(arr, np.float32)[sl], B)  # (ROWS,)
                for h in range(HALVES):
                    wdg[h * 128 + idx, k * 128 + idx] = rows[h * 128 + idx]
            m["wdiag"] = wdg
        in_maps.append(m)
    return in_maps


def kernel(drive, r, eps, beta, K_causal, steps):
    steps = int(steps)
    nc = get_nc(steps)
    in_maps = _host_prep(drive, r, eps, beta, K_causal)
    res = run_bass_kernel_spmd(nc, in_maps, list(range(N_CORES)))
    parts = [
        res.results[i]["out"].reshape(B, CPC, T).transpose(0, 2, 1)
        for i in range(N_CORES)
    ]
    return np.ascontiguousarray(np.concatenate(parts, axis=2), np.float32)


# revision 10
# speedup vs baseline: 1.0323x; 1.0323x over previous
"""Trainium2 Bass kernel for CausalSequenceCML.

Math (reference, per step, grid g laid out (B, C, T)):
    mapped  = r * g * (1 - g)
    local   = causal depthwise conv1d(mapped, K, left pad 3)   # per channel
    physics = (1 - eps) * mapped + eps * local
    g'      = (1 - beta) * physics + beta * x0                 # x0 = initial grid

Because r, eps, beta, K are per-channel constants and the conv is linear,
the whole update is affine in a = g*(1-g) = 0.25 - (g-0.5)^2:

    g' = D - C3*sq[t] - C2*sq[t-1] - C1*sq[t-2] - C0*sq[t-3]
    sq = (g - 0.5)^2
    Cj = (1-beta)*eps*r*K[j]             (j = 0, 1, 2)
    C3 = (1-beta)*r*((1-eps) + eps*K[3])
    D  = beta*x0 + 0.25*(C0+C1+C2+C3)

Left-boundary: conv pads mapped (=> a) with zeros, i.e. sq = 0.25 there; the
sq buffer has 3 leading pad columns held at 0.25.

Sharding: C=512 split across 8 cores (64 ch each). Per core the (B=4, 64, T)
block is flattened to 256 rows = 2 SBUF tiles of (128, 2+T), channels+batch
on partitions, time on the free dim (2 leading scratch cols, see below).

Engine split, per step per tile (all fp32 — the chaotic map amplifies
rounding ~3e4x over 16 steps, so 16-bit or float32r anywhere fails):
 - ScalarE: sq = Square(g - 0.5) into the padded sq buffer.
 - VectorE: columns [0, PE_SPLIT) via TWO custom-DVE FIR2 instructions
   (hand-authored uOp program FIR2_ANT):
       E  = D - C3*(sq[t]   + (C1/C3)*sq[t-2])
       g' = E - C2*(sq[t-1] + (C0/C2)*sq[t-3])
   Each runs at ~1 elem/cycle and computes 2 taps + merge: the even-shift
   tap reads the instruction's own input stream 2 elements back through the
   DVE datapath's cross-element flops (NEXT_ALU_OUT_B = next block's
   B-flop, which on TRN2 holds that block's result 2 elements earlier —
   measured on HW). The odd taps come from a second instruction whose
   input AP starts one column earlier. The first 2 outputs of each
   instruction are warmup garbage, absorbed by 2 leading scratch columns
   on the E/g buffers. This replaces 4 scalar_tensor_tensor ops (the
   per-element cost drops 4 -> 2 DVE cycles).
 - TensorE: columns [PE_SPLIT, T) via 5 PSUM-accumulated fp32 matmuls per
   512-col block: W = diag(-Cj) for the 4 taps (time shifts come free via
   the moving operand's AP column offset) plus an identity-diagonal matmul
   that adds D — so PSUM holds g' directly (fp32 matmul is 2-pass LO/HI,
   measured exact to 1e-7).
 - ScalarE copies the finished PSUM blocks to the state buffer.
GPSIMD stays idle: its SBUF port is an exclusive lock shared with DVE's
2-port ops, so concurrent GPSIMD work measured 2.8x slower overall.
"""

import copy

import numpy as np

from concourse import bacc, mybir
import concourse.tile as tile
import concourse.dve_ops as _dve_ops_mod
from concourse.bass_utils import run_bass_kernel_spmd
from concourse.dve_spec import Spec, Src0, Src1, C0 as _C0, C1 as _C1, lower as _dve_lower
from concourse.dve_uop import AluInp, AluOp, DelayInp, DveOpSpec

B, T, C = 4, 4096, 512
N_CORES = 8
CPC = C // N_CORES          # channels per core = 64
ROWS = B * CPC              # 256 rows per core
HALVES = ROWS // 128        # 2 SBUF tiles per core
CLAMP = 1e-4
F32 = mybir.dt.float32
PAD = 48                    # scratch lead cols on g/D/E (absorbs DVE pipe-fill skew)
SQPAD = PAD + 1             # sq lead cols (3 math pads + fill-warmup, all 0.25)

# PE offload: TensorE computes all 4 taps + D for columns [PE_SPLIT, T).
PE_SPLIT = 3264
PE_BLOCK = 512
PE_ADD_D = True
# DMA-D: DMA the D block into the PSUM bank each step and skip the 5th
# (identity-diagonal) matmul; the 4 tap matmuls then accumulate on top
# (start=False). Cuts PE cost per column 20 -> 16 cycles.
PE_DMA_D = False
# Emit the PE-region slice of the Square first (separate ACT op) so TensorE
# starts each step's matmuls ~2.7us earlier; the DVE-region slice follows.
SPLIT_SQ = False
# Fuse the PSUM->SBUF copy with the NEXT step's square: for s < steps-1 the
# PE-region square reads g' straight from the PSUM banks (ACT op), so the
# per-step copy only happens on the last step (where g' must materialize
# for the output DMA). Slims the ScalarE chain between steps.
PSUM_SQ = False
# Chunk the DVE region into two column halves, emitted right-half first
# (with the square split to match), so the FIR chain starts ~1.4us earlier
# each step. The right chunk's PAD-wide warmup head overwrites columns the
# left chunk later rewrites correctly (left emitted after right).
CHUNK = False

_compiled = {}


# --- custom DVE op: FIR2_ANT ------------------------------------------------
# out[k] = in1[k] - s0*(in0[k] + s1*in0[k-2])
# Built by hand at the uOp level (the Spec DSL has no delay primitive).
# Verified on HW: exact vs fp32 reference for all k >= 2.

class _HandDveOp:
    """Duck-types concourse.dve_ops.DveOp for _custom_dve + table-gen."""

    def __init__(self, name, spec, uops, rd1):
        self.name = name
        self.spec = spec
        self.uops = uops
        self.rd1 = rd1
        self.subdim = False

    def compile(self, ver):
        assert ver == "v3", "FIR2_ANT authored for TRN2/v3"
        return DveOpSpec(
            name=self.name,
            opcode=_dve_ops_mod.get_dve_sub_opcode(self.name),
            uops=list(self.uops),
            rd1_en=self.rd1,
        )


_fir2_op = None


def _get_fir2():
    global _fir2_op
    if _fir2_op is not None:
        return _fir2_op
    existing = next((o for o in _dve_ops_mod.OPS if o.name == "FIR2_ANT"), None)
    if existing is not None:
        _fir2_op = existing
        return existing
    spec = Spec(
        body=Src1 - (Src0 + Src0 * _C1) * _C0,
        reference=lambda in0, in1, s0, s1, imm2: in1 - (in0 + in0 * s1) * s0,
    )
    uops = _dve_lower(spec, ver="v3")
    assert len(uops) == 1
    u = copy.deepcopy(uops[0])
    chain = {}
    for j, s in enumerate(u.inp):
        if u.inp_enable[j]:
            chain[s.name] = j - 1
    x = AluInp.PREV_DELAY_0 + chain["SRC_0"]
    d = AluInp.PREV_DELAY_0 + chain["SRC_1"]
    c0 = AluInp.PREV_DELAY_0 + chain["CONST_0"]
    c1 = AluInp.PREV_DELAY_0 + chain["CONST_1"]
    used = set(chain.values())
    free = next(l for l in range(6) if l not in used)
    m_lane = AluInp.PREV_DELAY_0 + free

    dp = u.datapath_config
    # blk0: m = s1 * in0[k-2]  (NEXT_ALU_OUT_B = blk1's B-flop = blk1's
    # result 2 elements back)
    dp[0].op = AluOp.MULTIPLY
    dp[0].alu_src0 = AluInp.NEXT_ALU_OUT_B
    dp[0].alu_src1 = c1
    dp[0].alu_out_enable = 1
    # blk1: result = in0[k] (bypass); broadcast into B-flop; carry m on a
    # free delay lane
    dp[1].op = AluOp.BYPASS
    dp[1].alu_src0 = x
    dp[1].alu_src1 = x
    dp[1].alu_out_enable = 1
    dp[1].alu_out_b_enable = 1
    dp[1].delay[free] = DelayInp.PREV_ALU_OUT
    dp[1].delay_enable[free] = 1
    # blk2: u = in0[k] + m
    dp[2].op = AluOp.ADD
    dp[2].alu_src0 = AluInp.PREV_ALU_OUT
    dp[2].alu_src1 = m_lane
    dp[2].alu_out_enable = 1
    # blk3: v = s0 * u
    dp[3].op = AluOp.MULTIPLY
    dp[3].alu_src0 = AluInp.PREV_ALU_OUT
    dp[3].alu_src1 = c0
    dp[3].alu_out_enable = 1
    # blk4: out = in1 - v
    dp[4].op = AluOp.SUBTRACT
    dp[4].alu_src0 = d
    dp[4].alu_src1 = AluInp.PREV_ALU_OUT
    dp[4].alu_out_enable = 1
    for b in range(5, 8):
        dp[b].op = AluOp.BYPASS
        dp[b].alu_src0 = AluInp.PREV_ALU_OUT
        dp[b].alu_src1 = AluInp.PREV_ALU_OUT
        dp[b].alu_out_enable = 1
        dp[b].alu_out_a_enable = 0
        dp[b].alu_out_b_enable = 0

    op = _HandDveOp("FIR2_ANT", spec, (u,), rd1=True)
    _dve_ops_mod.OPS.append(op)
    row = _dve_ops_mod._CUSTOM_DVE_ROW_BASE + len(_dve_ops_mod.OPS) - 1
    assert row < 0x20
    _dve_ops_mod._SUB_OPCODE_FOR_NAME[op.name] = row
    _dve_ops_mod.CUSTOM_DVE_SPECS[op.name] = op.spec
    _fir2_op = op
    return op


# --- kernel build -----------------------------------------------------------

def _build(steps: int, loop_k: int | None = None, pe_split: int | None = None,
           pe_add_d: bool | None = None, pe_dma_d: bool | None = None,
           split_sq: bool | None = None, psum_sq: bool | None = None,
           chunk: bool | None = None):
    PE_SPLIT = globals()["PE_SPLIT"] if pe_split is None else pe_split
    PE_ADD_D = globals()["PE_ADD_D"] if pe_add_d is None else pe_add_d
    PE_DMA_D = globals()["PE_DMA_D"] if pe_dma_d is None else pe_dma_d
    SPLIT_SQ = globals()["SPLIT_SQ"] if split_sq is None else split_sq
    PSUM_SQ = globals()["PSUM_SQ"] if psum_sq is None else psum_sq
    CHUNK = globals()["CHUNK"] if chunk is None else chunk
    if PSUM_SQ:
        assert PE_ADD_D or PE_DMA_D  # psum must hold g' directly
    if PE_DMA_D:
        PE_ADD_D = False
    pe_on = PE_SPLIT < T
    fir2 = _get_fir2()
    nc = bacc.Bacc("TRN2", target_bir_lowering=False, debug=False)

    x = nc.dram_tensor("x", [ROWS, T], F32, kind="ExternalInput").ap()
    coef = nc.dram_tensor("coef", [ROWS, 6], F32, kind="ExternalInput").ap()
    out = nc.dram_tensor("out", [ROWS, T], F32, kind="ExternalOutput").ap()
    if pe_on:
        wcols = 640 if PE_ADD_D else 512
        wdiag = nc.dram_tensor("wdiag", [ROWS, wcols], F32,
                               kind="ExternalInput").ap()
        wdiag_h = wdiag.rearrange("(h p) c -> h p c", p=128)

    x_h = x.rearrange("(h p) t -> h p t", p=128)
    out_h = out.rearrange("(h p) t -> h p t", p=128)
    coef_h = coef.rearrange("(h p) c -> h p c", p=128)

    mult = mybir.AluOpType.mult
    add = mybir.AluOpType.add

    with tile.TileContext(nc) as tc:
        with tc.tile_pool(name="state", bufs=1) as pool, \
             tc.tile_pool(name="psum", bufs=8, space="PSUM") as pspool:
            neg_half = pool.tile([128, 1], F32, tag="neg_half", name="neg_half")
            nc.vector.memset(neg_half[:], -0.5)
            gA, gB, sq, D, E, cf = [], [], [], [], [], []
            for h in range(HALVES):
                gA.append(pool.tile([128, PAD + T], F32, tag=f"gA{h}", name=f"gA{h}"))
                gB.append(pool.tile([128, PAD + T], F32, tag=f"gB{h}", name=f"gB{h}"))
                sq.append(pool.tile([128, SQPAD + T], F32, tag=f"sq{h}", name=f"sq{h}"))
                D.append(pool.tile([128, PAD + T], F32, tag=f"D{h}", name=f"D{h}"))
                E.append(pool.tile([128, PAD + T], F32, tag=f"E{h}", name=f"E{h}"))
                cf.append(pool.tile([128, 6], F32, tag=f"cf{h}", name=f"cf{h}"))

            wd = []
            if pe_on:
                for h in range(HALVES):
                    wd.append(pool.tile([128, wcols], F32, tag=f"wd{h}",
                                        name=f"wd{h}"))
                    nc.sync.dma_start(out=wd[h][:], in_=wdiag_h[h])
            for h in range(HALVES):
                nc.sync.dma_start(out=cf[h][:], in_=coef_h[h])
                nc.sync.dma_start(out=gA[h][:, PAD:PAD + T], in_=x_h[h])
                # pad columns stay at a^2-of-zero = 0.25 forever
                nc.vector.memset(sq[h][:, 0:SQPAD], 0.25)
                # scratch lead cols: keep finite (warmup garbage lands here)
                nc.vector.memset(gA[h][:, 0:PAD], 0.0)
                nc.vector.memset(gB[h][:, 0:PAD], 0.0)
                nc.vector.memset(D[h][:, 0:PAD], 0.0)
                # D = beta * x0 + dconst
                nc.vector.tensor_scalar(
                    D[h][:, PAD:PAD + T], gA[h][:, PAD:PAD + T],
                    cf[h][:, 4:5], cf[h][:, 5:6], mult, add,
                )

            dve_end = PE_SPLIT if pe_on else T
            pe_blocks = []
            c = PE_SPLIT
            while c < T:
                n = min(PE_BLOCK, T - c)
                pe_blocks.append((c, n))
                c += n

            def emit_steps():
                prev_psums = []
                for s in range(steps):
                    cur, nxt = (gA, gB) if s % 2 == 0 else (gB, gA)
                    if CHUNK:
                        m = (dve_end // 2) & ~63
                        lo = max(m - PAD - 2, 0)
                        for h in range(HALVES):
                            nc.scalar.activation(
                                sq[h][:, SQPAD + lo:SQPAD + T],
                                cur[h][:, PAD + lo:PAD + T],
                                mybir.ActivationFunctionType.Square,
                                bias=neg_half[:],
                            )
                        for h in range(HALVES):
                            nc.scalar.activation(
                                sq[h][:, SQPAD:SQPAD + lo],
                                cur[h][:, PAD:PAD + lo],
                                mybir.ActivationFunctionType.Square,
                                bias=neg_half[:],
                            )
                    elif pe_on and PSUM_SQ:
                        # DVE region square from cur; PE region square
                        # straight from last step's PSUM banks (or from cur
                        # on the first step)
                        for h in range(HALVES):
                            end = T if s == 0 else dve_end
                            nc.scalar.activation(
                                sq[h][:, SQPAD:SQPAD + end],
                                cur[h][:, PAD:PAD + end],
                                mybir.ActivationFunctionType.Square,
                                bias=neg_half[:],
                            )
                        for (h, c0, n, ps) in prev_psums:
                            nc.scalar.activation(
                                sq[h][:, SQPAD + c0:SQPAD + c0 + n],
                                ps[:, :n],
                                mybir.ActivationFunctionType.Square,
                                bias=neg_half[:],
                            )
                    elif pe_on and SPLIT_SQ:
                        lo = dve_end - 3  # PE taps read sq cols [lo, T)
                        for h in range(HALVES):
                            nc.scalar.activation(
                                sq[h][:, SQPAD + lo:SQPAD + T],
                                cur[h][:, PAD + lo:PAD + T],
                                mybir.ActivationFunctionType.Square,
                                bias=neg_half[:],
                            )
                        for h in range(HALVES):
                            nc.scalar.activation(
                                sq[h][:, SQPAD:SQPAD + lo],
                                cur[h][:, PAD:PAD + lo],
                                mybir.ActivationFunctionType.Square,
                                bias=neg_half[:],
                            )
                    else:
                        for h in range(HALVES):
                            nc.scalar.activation(
                                sq[h][:, SQPAD:SQPAD + T], cur[h][:, PAD:PAD + T],
                                mybir.ActivationFunctionType.Square,
                                bias=neg_half[:],
                            )
                    # PE region: psum accumulates -Cj taps (+D), ScalarE
                    # copies g' out
                    step_psums = []
                    if pe_on:
                        n_mm = 5 if PE_ADD_D else 4
                        for h in range(HALVES):
                            for (c0, n) in pe_blocks:
                                ps = pspool.tile([128, PE_BLOCK], F32, tag="ps",
                                                 name=f"ps{s}_{h}_{c0}")
                                if PE_DMA_D:
                                    # ACT seeds the bank with D; taps then
                                    # accumulate on top (start=False)
                                    nc.scalar.copy(
                                        ps[:, :n],
                                        D[h][:, PAD + c0:PAD + c0 + n],
                                    )
                                for k in range(4):
                                    off = SQPAD - k
                                    # DMA_D: each tap is its own
                                    # start=False/stop=True group — a
                                    # multi-member start=False group
                                    # drops the seeded bank content
                                    # (measured on HW)
                                    nc.tensor.matmul(
                                        ps[:, :n],
                                        wd[h][:, k * 128:(k + 1) * 128],
                                        sq[h][:, off + c0:off + c0 + n],
                                        start=(k == 0 and not PE_DMA_D),
                                        stop=(PE_DMA_D or k == n_mm - 1),
                                        skip_group_check=PE_DMA_D,
                                    )
                                if PE_ADD_D:
                                    nc.tensor.matmul(
                                        ps[:, :n], wd[h][:, 512:640],
                                        D[h][:, PAD + c0:PAD + c0 + n],
                                        start=False, stop=True,
                                    )
                                step_psums.append((h, c0, n, ps))
                    # DVE region: two FIR2 ops per tile.
                    # Output position j <-> real col t = j - PAD; the first
                    # ~25 outputs are pipe-fill garbage (tap offset settles
                    # to exactly 2-back only after the fill phase), absorbed
                    # by PAD scratch lead cols; lead pads are constant 0.25
                    # so skewed warm-up taps read identical values.
                    if CHUNK:
                        m = (dve_end // 2) & ~63
                        ranges = [(m, dve_end), (0, m)]  # right first
                    else:
                        ranges = [(0, dve_end)]
                    for (a, b) in ranges:
                        # ops over output cols [a, b): out AP starts at col
                        # a with a PAD warmup head landing in [a, a+PAD) —
                        # scratch when a=0, else overwritten by the later
                        # left chunk.
                        wlen = PAD + (b - a)
                        for h in range(HALVES):
                            # E = D - C3*(sq[t] + (C1/C3)*sq[t-2])
                            nc.vector._custom_dve(
                                fir2, out=E[h][:, a:a + wlen],
                                in0=sq[h][:, a + 1:a + 1 + wlen],
                                in1=D[h][:, a:a + wlen],
                                s0=cf[h][:, 0:1], s1=cf[h][:, 1:2],
                            )
                            # g' = E - C2*(sq[t-1] + (C0/C2)*sq[t-3])
                            nc.vector._custom_dve(
                                fir2, out=nxt[h][:, a:a + wlen],
                                in0=sq[h][:, a:a + wlen],
                                in1=E[h][:, a:a + wlen],
                                s0=cf[h][:, 2:3], s1=cf[h][:, 3:4],
                            )
                    for (h, c0, n, ps) in step_psums:
                        if PSUM_SQ and s < steps - 1:
                            continue  # next step squares from psum directly
                        if PE_ADD_D or PE_DMA_D:
                            nc.scalar.copy(nxt[h][:, PAD + c0:PAD + c0 + n],
                                           ps[:, :n])
                        else:
                            nc.vector.scalar_tensor_tensor(
                                nxt[h][:, PAD + c0:PAD + c0 + n], ps[:, :n],
                                -1.0, D[h][:, PAD + c0:PAD + c0 + n],
                                mult, add,
                            )
                    prev_psums = step_psums

            if loop_k is not None:
                with tc.For_i(0, loop_k):
                    emit_steps()
            else:
                emit_steps()

            fin = gA if steps % 2 == 0 else gB
            for h in range(HALVES):
                nc.vector.tensor_scalar(
                    fin[h][:, PAD:PAD + T], fin[h][:, PAD:PAD + T],
                    CLAMP, 1.0 - CLAMP,
                    mybir.AluOpType.max, mybir.AluOpType.min,
                )
                nc.sync.dma_start(out=out_h[h], in_=fin[h][:, PAD:PAD + T])

    nc.compile()
    return nc


def get_nc(steps: int):
    if steps not in _compiled:
        _compiled[steps] = _build(steps)
    return _compiled[steps]


def _host_prep(drive, r, eps, beta, K_causal):
    """Per-core input maps: x (256, T), coef (256, 6), wdiag (256, 640)."""
    drive = np.asarray(drive, np.float32)
    r = np.asarray(r, np.float32)
    eps = np.asarray(eps, np.float32)
    beta = np.asarray(beta, np.float32)
    K = np.asarray(K_causal, np.float32)[:, 0, :]  # (C, 4)

    one_m_b = 1.0 - beta
    C0 = one_m_b * eps * r * K[:, 0]
    C1 = one_m_b * eps * r * K[:, 1]
    C2 = one_m_b * eps * r * K[:, 2]
    C3 = one_m_b * r * ((1.0 - eps) + eps * K[:, 3])
    dconst = 0.25 * (C0 + C1 + C2 + C3)

    pe_on = PE_SPLIT < T
    in_maps = []
    idx = np.arange(128)
    for i in range(N_CORES):
        sl = slice(i * CPC, (i + 1) * CPC)
        xs = np.ascontiguousarray(
            drive[:, :, sl].transpose(0, 2, 1).reshape(ROWS, T), np.float32
        )
        cs = np.stack(
            [np.tile(C3[sl], B), np.tile(C1[sl] / C3[sl], B),
             np.tile(C2[sl], B), np.tile(C0[sl] / C2[sl], B),
             np.tile(beta[sl], B), np.tile(dconst[sl], B)],
            axis=1,
        ).astype(np.float32)
        m = {"x": xs, "coef": np.ascontiguousarray(cs)}
        if pe_on:
            sign = -1.0 if (PE_ADD_D or PE_DMA_D) else 1.0
            blocks = [sign * C3, sign * C2, sign * C1, sign * C0]
            if PE_ADD_D:
                blocks.append(np.ones(C, np.float32))
            wdg = np.zeros((ROWS, 128 * len(blocks)), np.float32)
            for k, arr in enumerate(blocks):
                rows = np.tile(np.asarray(arr, np.float32)[sl], B)  # (ROWS,)
                for h in range(HALVES):
                    wdg[h * 128 + idx, k * 128 + idx] = rows[h * 128 + idx]
            m["wdiag"] = wdg
        in_maps.append(m)
    return in_maps


def kernel(drive, r, eps, beta, K_causal, steps):
    steps = int(steps)
    nc = get_nc(steps)
    in_maps = _host_prep(drive, r, eps, beta, K_causal)
    res = run_bass_kernel_spmd(nc, in_maps, list(range(N_CORES)))
    parts = [
        res.results[i]["out"].reshape(B, CPC, T).transpose(0, 2, 1)
        for i in range(N_CORES)
    ]
    return np.ascontiguousarray(np.concatenate(parts, axis=2), np.float32)
